# revision 12
# baseline (speedup 1.0000x reference)
"""Trainium2 Bass kernel for a 3-layer GCN (nn_BaselineGCN).

Strategy (8 NeuronCores, node partitioning by dst):
  - Host: compute deg/dis, partition edges by owner of dst (6250 nodes/core,
    padded to 6272), order edges by (4-window block, src-half, window,
    ascending src index), build int16 gather indices (full node table split
    into two halves so indices fit int16) plus per-edge local-dst values for
    the one-hot construction.  Slot padding inside a (block, half) section
    indexes row 0 (harmless; one-hot zeroes it); padding AFTER the section's
    last real edge is -1 so the SWDGE ucode trims it for free.
  - Device, per layer (bf16 tables, fp32 accumulation):
      * data-parallel matmul  Zs_own = dis (H_own @ W)          (TensorE)
      * AllGather Zs chunks -> full 50176-row bf16 table in HBM (collective)
      * per 4-window block: gather calls of up to 8x128 indices that run
        ACROSS window boundaries within each (block, half) section (the
        SWDGE ucode caps one call at 1024 indices; merging sections cuts the
        ~1us/call fixed descriptor-generation cost on GpSimd, the kernel's
        bottleneck), then per 128-dst window: one-hot(dst_local) built on DVE
        via is_equal vs iota, segment-sum via PE matmul accumulation in PSUM;
        the (A+I) self term is one extra matmul I @ Zs_own[w] in the same
        PSUM chain; epilogue relu(dis * acc), TensorE transpose ->
        next H^T kept resident in SBUF.
  - Layer 3 (64 outputs) runs on a 128-wide bf16 table (upper 64 cols
    garbage, excluded by slicing) so the whole edge path is uniform bf16.
"""
import sys
import os

sys.path.insert(0, "/opt/trn_rl_repo")

import numpy as np

NC_CORES = 8
W_BLK = 4   # dst windows per gather block
GMAX = 8    # max groups (=1024 indices) per dma_gather call (ucode limit)


def _cdiv(a, b):
    return (a + b - 1) // b


# ---------------------------------------------------------------------------
# Host-side preprocessing
# ---------------------------------------------------------------------------
def preprocess(edge_index, N):
    src = np.asarray(edge_index[0], dtype=np.int64)
    dst = np.asarray(edge_index[1], dtype=np.int64)
    deg = np.bincount(dst, minlength=N).astype(np.float32) + np.float32(1.0)
    dis = (np.float32(1.0) / np.sqrt(deg)).astype(np.float32)

    CH = N // NC_CORES
    NWIN = _cdiv(CH, 128)
    CHP = NWIN * 128
    # split each core's chunk into A (windows 0..NWA-1) and B (the rest) so
    # int16 gather indices can address each half-table (NC*HA <= 32768).
    NWA = min(NWIN - 1, 32768 // (NC_CORES * 128)) if NWIN > 1 else NWIN
    HA = NWA * 128            # rows per core in table A
    HB = CHP - HA             # rows per core in table B
    src_c = src // CH         # owning core of each src node
    src_o = src % CH          # offset within core

    counts = np.zeros((NC_CORES, NWIN, 2), dtype=np.int64)
    percore = []
    for c in range(NC_CORES):
        sel = (dst >= c * CH) & (dst < (c + 1) * CH)
        sc, so = src_c[sel], src_o[sel]
        ed = dst[sel] - c * CH
        w = ed >> 7
        blk = w // W_BLK
        h = (so >= HA).astype(np.int64)
        eidx = np.where(h == 0, sc * HA + so, sc * HB + (so - HA))
        # stream order: block -> half -> window -> ascending src index
        # (ascending gather addresses improve HBM locality)
        order = np.lexsort((eidx, w, h, blk))
        eidx, ed, w, h = eidx[order], ed[order], w[order], h[order]
        np.add.at(counts[c], (w, h), 1)
        percore.append((eidx, ed))

    G = _cdiv(counts, 128).max(axis=0)  # [NWIN, 2] groups per (window, half)

    import ml_dtypes

    NBLK = _cdiv(NWIN, W_BLK)
    cores = []
    for c in range(NC_CORES):
        eidx, ed = percore[c]
        idx_parts, dstl_parts = [], []
        pos = 0
        for b in range(NBLK):
            ws = range(b * W_BLK, min((b + 1) * W_BLK, NWIN))
            for hi in range(2):
                sec_idx, sec_dstl = [], []
                for wi in ws:
                    n = counts[c, wi, hi]
                    g = G[wi, hi]
                    seg_idx = np.zeros(g * 128, dtype=np.int16)  # pad: row 0
                    seg_dstl = np.full(g * 128, 255.0, dtype=np.float32)
                    if n:
                        seg_idx[:n] = eidx[pos:pos + n].astype(np.int16)
                        seg_dstl[:n] = (ed[pos:pos + n] - wi * 128).astype(
                            np.float32)
                        pos += n
                    sec_idx.append(seg_idx)
                    sec_dstl.append(seg_dstl)
                if sec_idx:
                    # pad slots keep index 0 (a valid row; their one-hot
                    # weight is 0) so every call is a full static-size
                    # gather: num_idxs_reg == num_idxs, no count registers
                    idx_parts.append(np.concatenate(sec_idx))
                    dstl_parts.append(np.concatenate(sec_dstl))
        idx_all = np.concatenate(idx_parts)
        dstl_all = np.concatenate(dstl_parts)
        TOT_G = len(idx_all) // 128

        # device layouts
        idx_tiled = np.tile(idx_all.reshape(-1, 16).T, (8, 1)).copy()
        # dstl: [128 edge-slot, TOT_G] bf16
        dstl_tiled = np.ascontiguousarray(
            dstl_all.reshape(TOT_G, 128).T).astype(ml_dtypes.bfloat16)
        d = np.ones(CHP, np.float32)
        d[:CH] = dis[c * CH:(c + 1) * CH]
        dis_win = np.ascontiguousarray(d.reshape(NWIN, 128).T)
        cores.append(dict(idx=idx_tiled, dstl=dstl_tiled, dis_win=dis_win))
    return G, cores, CH, NWIN, CHP, NWA


# ---------------------------------------------------------------------------
# Bass program
# ---------------------------------------------------------------------------
def build_program(DIN, DRS, DTS, G, NWIN, CHP, NWA, TOT_IDX, TOT_G,
                  G_CAP, BLK_CAP, biases_nonzero):
    """DRS: real per-layer output dims [256,256,64];
    DTS: padded table dims [256,256,128]."""
    from concourse import bacc, bass, tile, mybir

    f32 = mybir.dt.float32
    bf16 = mybir.dt.bfloat16
    i16 = mybir.dt.int16
    ADD = mybir.AluOpType.add
    EQ = mybir.AluOpType.is_equal
    CPY = mybir.ActivationFunctionType.Copy
    NL = len(DRS)
    NBLK = _cdiv(NWIN, W_BLK)

    nc = bacc.Bacc("TRN2", target_bir_lowering=False, debug=False,
                   enable_asserts=False, num_devices=NC_CORES,
                   num_swdge_queues=4, dynamic_dma_scratch_size=32768)

    # --- I/O tensors ---
    xT_d = nc.dram_tensor("xT", [DIN, CHP], bf16, kind="ExternalInput")
    W_d = [nc.dram_tensor(f"W{i}", [DRS[i - 1] if i else DIN, DRS[i]], bf16,
                          kind="ExternalInput") for i in range(NL)]
    bias_d = [nc.dram_tensor(f"bias{i}", [128, DRS[i]], f32,
                             kind="ExternalInput") for i in range(NL)]
    idx_d = nc.dram_tensor("idx", [128, TOT_IDX // 16], i16, kind="ExternalInput")
    dstl_d = nc.dram_tensor("dstl", [128, TOT_G], bf16, kind="ExternalInput")
    iotag_d = nc.dram_tensor("iotag", [128, 128 * G_CAP], bf16,
                             kind="ExternalInput")
    dis_d = nc.dram_tensor("dis_win", [128, NWIN], f32, kind="ExternalInput")
    ident_d = nc.dram_tensor("ident", [128, 128], bf16, kind="ExternalInput")
    out_d = nc.dram_tensor("out", [CHP, DRS[-1]], f32, kind="ExternalOutput")

    with tile.TileContext(nc) as tc:
        with (
            tc.tile_pool(name="const", bufs=1) as constp,
            tc.tile_pool(name="ht", bufs=1) as htp,
            tc.tile_pool(name="wts", bufs=2) as wtsp,
            tc.tile_pool(name="zs", bufs=3) as zsp,
            tc.tile_pool(name="zw", bufs=3) as zwp,
            tc.tile_pool(name="gath", bufs=2) as gathp,
            tc.tile_pool(name="ohA", bufs=3) as ohpA,
            tc.tile_pool(name="ohB", bufs=3) as ohpB,
            tc.tile_pool(name="epi", bufs=3) as epip,
            tc.tile_pool(name="psz", bufs=2, space="PSUM") as pszp,
            tc.tile_pool(name="psw", bufs=3, space="PSUM") as pswp,
            tc.tile_pool(name="pst", bufs=2, space="PSUM") as pstp,
            tc.tile_pool(name="dram", bufs=1, space="DRAM") as dramp,
        ):
            # --- persistent SBUF constants ---
            idx_t = constp.tile([128, TOT_IDX // 16], i16, tag="idx")
            nc.sync.dma_start(idx_t[:], idx_d[:])
            dstl_t = constp.tile([128, TOT_G], bf16, tag="dstl")
            nc.sync.dma_start(dstl_t[:], dstl_d[:])
            iotag_t = constp.tile([128, 128 * G_CAP], bf16, tag="iotag")
            nc.sync.dma_start(iotag_t[:], iotag_d[:])
            dis_t = constp.tile([128, NWIN], f32, tag="dis")
            nc.sync.dma_start(dis_t[:], dis_d[:])
            ident_t = constp.tile([128, 128], bf16, tag="ident")
            nc.sync.dma_start(ident_t[:], ident_d[:])
            # one gpsimd register per distinct gather size, set once
            sizes = set()
            for b in range(NBLK):
                for hh in range(2):
                    gsec = int(G[b * W_BLK:(b + 1) * W_BLK, hh].sum())
                    while gsec > 0:
                        gc = min(GMAX, gsec)
                        sizes.add(gc * 128)
                        gsec -= gc
            nreg = {}
            for v in sorted(sizes):
                r = nc.gpsimd.alloc_register(f"nidx{v}")
                nc.gpsimd.reg_mov(r, v)
                nreg[v] = r
            bias_t = []
            for i in range(NL):
                if biases_nonzero[i]:
                    bt = constp.tile([128, DRS[i]], f32, tag=f"bias{i}")
                    nc.sync.dma_start(bt[:], bias_d[i][:])
                    bias_t.append(bt)
                else:
                    bias_t.append(None)

            # --- H^T SBUF-resident double buffer: [k][128, CHP] bf16 ---
            KT0 = DIN // 128
            ht_cur = [htp.tile([128, CHP], bf16, tag=f"htA{k}",
                               name=f"htA{k}") for k in range(KT0)]
            for k in range(KT0):
                nc.sync.dma_start(ht_cur[k][:], xT_d[k * 128:(k + 1) * 128, :])
            ht_nxt = [htp.tile([128, CHP], bf16, tag=f"htB{k}",
                               name=f"htB{k}") for k in range(KT0)]

            zs_own = [dramp.tile([CHP, DTS[i]], bf16, tag=f"zso{i}",
                                 name=f"zs_own{i}") for i in range(NL)]
            HA = NWA * 128
            HB = CHP - HA
            zs_fullA = [dramp.tile([NC_CORES * HA, DTS[i]], bf16,
                                   tag=f"zsfA{i}", addr_space="Shared",
                                   name=f"zs_fullA{i}") for i in range(NL)]
            zs_fullB = [dramp.tile([NC_CORES * HB, DTS[i]], bf16,
                                   tag=f"zsfB{i}", addr_space="Shared",
                                   name=f"zs_fullB{i}") for i in range(NL)]

            RG = [list(range(NC_CORES))]

            def emit_z_tile(li, t, lhs_tiles):
                """Z-matmul + scale for node-tile t of layer li."""
                Dr = DRS[li]
                Dt = DTS[li]
                KT = DIN // 128 if li == 0 else DRS[li - 1] // 128
                psz = pszp.tile([128, Dr], f32, tag="psz", name="psz")
                for k in range(KT):
                    nc.tensor.matmul(psz[:],
                                     lhs_tiles[k][:, t * 128:(t + 1) * 128],
                                     wk[li][k][:],
                                     start=(k == 0), stop=(k == KT - 1))
                zst = zsp.tile([128, Dt], bf16, tag="zst", name="zst")
                nc.scalar.activation(zst[:, :Dr], psz[:], CPY,
                                     scale=dis_t[:, t:t + 1])
                if Dr < Dt:
                    nc.vector.memset(zst[:, Dr:], 0.0)
                nc.sync.dma_start(zs_own[li][t * 128:(t + 1) * 128, :],
                                  zst[:])

            def emit_ag(li, half):
                if half == 0:
                    nc.gpsimd.collective_compute(
                        "AllGather", bass.mybir.AluOpType.bypass,
                        replica_groups=RG,
                        ins=[zs_own[li][:HA, :]],
                        outs=[zs_fullA[li].opt()])
                else:
                    nc.gpsimd.collective_compute(
                        "AllGather", bass.mybir.AluOpType.bypass,
                        replica_groups=RG,
                        ins=[zs_own[li][HA:, :]],
                        outs=[zs_fullB[li].opt()])

            # weight tiles for every layer, loaded up front (small)
            wk = []
            for li in range(NL):
                KT = DIN // 128 if li == 0 else DRS[li - 1] // 128
                wkl = []
                for k in range(KT):
                    wt_ = wtsp.tile([128, DRS[li]], bf16, tag=f"wk{li}_{k}",
                                    name=f"wk{li}_{k}")
                    nc.sync.dma_start(wt_[:], W_d[li][k * 128:(k + 1) * 128, :])
                    wkl.append(wt_)
                wk.append(wkl)

            # ---- layer-0 z-phase + split AllGather ----
            for t in range(NWIN):
                emit_z_tile(0, t, ht_cur)
                if t == NWA - 1:
                    emit_ag(0, 0)
            emit_ag(0, 1)

            # ---- main loop: gather/aggregate layer li; z + AG of li+1
            #      interleaved so the collectives hide under the gathers ----
            for li in range(NL):
                Dr = DRS[li]
                Dt = DTS[li]
                ioff16 = 0   # running idx offset (int16 cols)
                gbase = 0    # running group (dstl column) offset
                qrr = 0
                for b in range(NBLK):
                    ws = list(range(b * W_BLK, min((b + 1) * W_BLK, NWIN)))
                    Gls = [int(G[w, 0]) for w in ws]
                    Ghs = [int(G[w, 1]) for w in ws]
                    AGb, BGb = sum(Gls), sum(Ghs)
                    TG = AGb + BGb
                    wt = gathp.tile([128, BLK_CAP, Dt], bf16, tag="gather",
                                    name="wt")
                    for half, gsec, goff in ((0, AGb, 0), (1, BGb, AGb)):
                        tbl = zs_fullA[li] if half == 0 else zs_fullB[li]
                        g0 = 0
                        while g0 < gsec:
                            gc = min(GMAX, gsec - g0)
                            nc.gpsimd.dma_gather(
                                wt[:, goff + g0:goff + g0 + gc, :],
                                tbl[:],
                                idx_t[:, ioff16:ioff16 + gc * 8],
                                num_idxs=gc * 128,
                                num_idxs_reg=nreg[gc * 128],
                                elem_size=Dt,
                                queue_num=qrr % 4)
                            qrr += 1
                            ioff16 += gc * 8
                            g0 += gc
                    aoff, boff = 0, 0
                    for wi_i, w in enumerate(ws):
                        Gl, Gh = Gls[wi_i], Ghs[wi_i]
                        # one-hot per half: oh[p, j, g] = (dstl[p, g] == j)
                        ohA = None
                        if Gl:
                            ohA = ohpA.tile([128, 128, Gl], bf16, tag="ohA",
                                            name="ohA")
                            nc.vector.tensor_tensor(
                                ohA[:],
                                dstl_t[:, gbase + aoff:gbase + aoff + Gl]
                                    .unsqueeze(1).broadcast_to((128, 128, Gl)),
                                iotag_t[:].rearrange("p (j g) -> p j g",
                                                     g=G_CAP)[:, :, :Gl],
                                op=EQ,
                            )
                        ohB = None
                        if Gh:
                            ohB = ohpB.tile([128, 128, Gh], bf16, tag="ohB",
                                            name="ohB")
                            nc.vector.tensor_tensor(
                                ohB[:],
                                dstl_t[:, gbase + AGb + boff:
                                       gbase + AGb + boff + Gh]
                                    .unsqueeze(1).broadcast_to((128, 128, Gh)),
                                iotag_t[:].rearrange("p (j g) -> p j g",
                                                     g=G_CAP)[:, :, :Gh],
                                op=EQ,
                            )
                        # z rows for the self term (bf16, local DRAM)
                        zw = zwp.tile([128, Dr], bf16, tag="zw", name="zw")
                        nc.sync.dma_start(
                            zw[:], zs_own[li][w * 128:(w + 1) * 128, :Dr])
                        psw = pswp.tile([128, Dt], f32, tag="psw", name="psw")
                        nmm = Gl + Gh + 1
                        k = 0
                        for g in range(Gl):
                            nc.tensor.matmul(psw[:], ohA[:, :, g],
                                             wt[:, aoff + g, :],
                                             start=(k == 0), stop=False)
                            k += 1
                        for g in range(Gh):
                            nc.tensor.matmul(psw[:], ohB[:, :, g],
                                             wt[:, AGb + boff + g, :],
                                             start=(k == 0), stop=False)
                            k += 1
                        # self term: psw[j] += zs_own[w*128+j]  (the
                        # epilogue's dis[dst] scale turns this into the
                        # dis^2 z self contribution)
                        nc.tensor.matmul(psw[:, :Dr], ident_t[:], zw[:],
                                         start=(k == 0), stop=True)
                        aoff += Gl
                        boff += Gh
                        # epilogue: relu(dis * acc) (+bias if nonzero)
                        if bias_t[li] is not None:
                            t1 = epip.tile([128, Dr], f32, tag="t1",
                                           name="t1")
                            nc.vector.tensor_tensor(t1[:], psw[:, :Dr],
                                                    bias_t[li][:], op=ADD)
                            src_ap = t1[:]
                        else:
                            src_ap = psw[:, :Dr]
                        if li < NL - 1:
                            h2 = epip.tile([128, Dr], bf16, tag="h2",
                                           name="h2")
                            nc.scalar.activation(
                                h2[:], src_ap,
                                bass.mybir.ActivationFunctionType.Relu,
                                scale=dis_t[:, w:w + 1])
                            for k2 in range(Dr // 128):
                                pst = pstp.tile([128, 128], bf16, tag="pst",
                                                name="pst")
                                nc.tensor.transpose(
                                    pst[:], h2[:, k2 * 128:(k2 + 1) * 128],
                                    ident_t[:])
                                nc.vector.tensor_copy(
                                    ht_nxt[k2][:, w * 128:(w + 1) * 128],
                                    pst[:])
                            # interleaved z for layer li+1 (its H^T tile-w is
                            # ready now); AG halves fire mid-phase
                            emit_z_tile(li + 1, w, ht_nxt)
                            if w == NWA - 1:
                                emit_ag(li + 1, 0)
                        else:
                            h2o = epip.tile([128, Dr], f32, tag="h2o",
                                            name="h2o")
                            nc.scalar.activation(h2o[:], src_ap, CPY,
                                                 scale=dis_t[:, w:w + 1])
                            nc.sync.dma_start(out_d[w * 128:(w + 1) * 128, :],
                                              h2o[:])
                    gbase += TG
                if li < NL - 1:
                    emit_ag(li + 1, 1)
                    ht_cur, ht_nxt = ht_nxt, ht_cur
    nc.compile()
    return nc


# ---------------------------------------------------------------------------
# Entry point
# ---------------------------------------------------------------------------
def kernel(x, edge_index, W1, b1, W2, b2, W3, b3):
    from concourse.bass_utils import run_bass_kernel_spmd
    import ml_dtypes

    bfnp = ml_dtypes.bfloat16
    x = np.asarray(x, dtype=np.float32)
    Ws = [np.asarray(w, dtype=np.float32) for w in (W1, W2, W3)]
    bs = [np.asarray(b, dtype=np.float32) for b in (b1, b2, b3)]

    N, DIN = x.shape
    DRS = [w.shape[1] for w in Ws]
    DTS = [max(d, 128) for d in DRS]
    NL = 3

    G, cores, CH, NWIN, CHP, NWA = preprocess(edge_index, N)
    TOT_IDX = cores[0]["idx"].shape[1] * 16
    TOT_G = cores[0]["dstl"].shape[1]
    G_CAP = int(G.max())
    NBLK = _cdiv(NWIN, W_BLK)
    BLK_CAP = max(
        int(G[b * W_BLK:(b + 1) * W_BLK].sum()) for b in range(NBLK))
    biases_nonzero = [bool(np.any(b != 0)) for b in bs]

    nc = build_program(DIN, DRS, DTS, G, NWIN, CHP, NWA, TOT_IDX, TOT_G,
                       G_CAP, BLK_CAP, biases_nonzero)

    ident = np.eye(128, dtype=bfnp)
    # iotag[p, j*G_CAP + g] = j
    iotag = np.tile(np.repeat(np.arange(128), G_CAP).astype(bfnp), (128, 1))
    in_maps = []
    for c in range(NC_CORES):
        xT = np.zeros((DIN, CHP), bfnp)
        xT[:, :CH] = x[c * CH:(c + 1) * CH].T.astype(bfnp)
        m = {
            "xT": xT,
            "idx": cores[c]["idx"],
            "dstl": cores[c]["dstl"],
            "iotag": iotag,
            "dis_win": cores[c]["dis_win"],
            "ident": ident,
        }
        for i in range(NL):
            m[f"W{i}"] = Ws[i].astype(bfnp)
            m[f"bias{i}"] = np.tile(bs[i][None, :], (128, 1))
        in_maps.append(m)

    trace = bool(int(os.environ.get("GCN_TRACE", "0")))
    res = run_bass_kernel_spmd(nc, in_maps, core_ids=list(range(NC_CORES)),
                               trace=trace)
    kernel.last_results = res
    out = np.concatenate([res.results[c]["out"][:CH] for c in range(NC_CORES)],
                         axis=0)
    return out.astype(np.float32)


# revision 13
# speedup vs baseline: 1.0169x; 1.0169x over previous
"""Trainium2 Bass kernel for a 3-layer GCN (nn_BaselineGCN).

Strategy (8 NeuronCores, node partitioning by dst):
  - Host: compute deg/dis, partition edges by owner of dst (6250 nodes/core,
    padded to 6272), order edges by (4-window block, src-half, window,
    ascending src index), build int16 gather indices (full node table split
    into two halves so indices fit int16) plus per-edge local-dst values for
    the one-hot construction.  Slot padding inside a (block, half) section
    indexes row 0 (harmless; one-hot zeroes it); padding AFTER the section's
    last real edge is -1 so the SWDGE ucode trims it for free.
  - Device, per layer (bf16 tables, fp32 accumulation):
      * data-parallel matmul  Zs_own = dis (H_own @ W)          (TensorE)
      * AllGather Zs chunks -> full 50176-row bf16 table in HBM (collective)
      * per 4-window block: gather calls of up to 8x128 indices that run
        ACROSS window boundaries within each (block, half) section (the
        SWDGE ucode caps one call at 1024 indices; merging sections cuts the
        ~1us/call fixed descriptor-generation cost on GpSimd, the kernel's
        bottleneck), then per 128-dst window: one-hot(dst_local) built on DVE
        via is_equal vs iota, segment-sum via PE matmul accumulation in PSUM;
        the (A+I) self term is one extra matmul I @ Zs_own[w] in the same
        PSUM chain; epilogue relu(dis * acc), TensorE transpose ->
        next H^T kept resident in SBUF.
  - Layer 3 (64 outputs) runs on a 128-wide bf16 table (upper 64 cols
    garbage, excluded by slicing) so the whole edge path is uniform bf16.
"""
import sys
import os

sys.path.insert(0, "/opt/trn_rl_repo")

import numpy as np

NC_CORES = 8
W_BLK = 2   # dst windows per gather block
GMAX = 8    # max groups (=1024 indices) per dma_gather call (ucode limit)


def _cdiv(a, b):
    return (a + b - 1) // b


# ---------------------------------------------------------------------------
# Host-side preprocessing
# ---------------------------------------------------------------------------
def preprocess(edge_index, N):
    src = np.asarray(edge_index[0], dtype=np.int64)
    dst = np.asarray(edge_index[1], dtype=np.int64)
    deg = np.bincount(dst, minlength=N).astype(np.float32) + np.float32(1.0)
    dis = (np.float32(1.0) / np.sqrt(deg)).astype(np.float32)

    CH = N // NC_CORES
    NWIN = _cdiv(CH, 128)
    CHP = NWIN * 128
    # split each core's chunk into A (windows 0..NWA-1) and B (the rest) so
    # int16 gather indices can address each half-table (NC*HA <= 32768).
    NWA = min(NWIN - 1, 32768 // (NC_CORES * 128)) if NWIN > 1 else NWIN
    HA = NWA * 128            # rows per core in table A
    HB = CHP - HA             # rows per core in table B
    src_c = src // CH         # owning core of each src node
    src_o = src % CH          # offset within core

    counts = np.zeros((NC_CORES, NWIN, 2), dtype=np.int64)
    percore = []
    for c in range(NC_CORES):
        sel = (dst >= c * CH) & (dst < (c + 1) * CH)
        sc, so = src_c[sel], src_o[sel]
        ed = dst[sel] - c * CH
        w = ed >> 7
        blk = w // W_BLK
        h = (so >= HA).astype(np.int64)
        eidx = np.where(h == 0, sc * HA + so, sc * HB + (so - HA))
        # stream order: block -> half -> window -> ascending src index
        # (ascending gather addresses improve HBM locality)
        order = np.lexsort((eidx, w, h, blk))
        eidx, ed, w, h = eidx[order], ed[order], w[order], h[order]
        np.add.at(counts[c], (w, h), 1)
        percore.append((eidx, ed))

    G = _cdiv(counts, 128).max(axis=0)  # [NWIN, 2] groups per (window, half)

    import ml_dtypes

    NBLK = _cdiv(NWIN, W_BLK)
    cores = []
    for c in range(NC_CORES):
        eidx, ed = percore[c]
        idx_parts, dstl_parts = [], []
        pos = 0
        for b in range(NBLK):
            ws = range(b * W_BLK, min((b + 1) * W_BLK, NWIN))
            for hi in range(2):
                sec_idx, sec_dstl = [], []
                for wi in ws:
                    n = counts[c, wi, hi]
                    g = G[wi, hi]
                    seg_idx = np.zeros(g * 128, dtype=np.int16)  # pad: row 0
                    seg_dstl = np.full(g * 128, 255.0, dtype=np.float32)
                    if n:
                        seg_idx[:n] = eidx[pos:pos + n].astype(np.int16)
                        seg_dstl[:n] = (ed[pos:pos + n] - wi * 128).astype(
                            np.float32)
                        pos += n
                    sec_idx.append(seg_idx)
                    sec_dstl.append(seg_dstl)
                if sec_idx:
                    # pad slots keep index 0 (a valid row; their one-hot
                    # weight is 0) so every call is a full static-size
                    # gather: num_idxs_reg == num_idxs, no count registers
                    idx_parts.append(np.concatenate(sec_idx))
                    dstl_parts.append(np.concatenate(sec_dstl))
        idx_all = np.concatenate(idx_parts)
        dstl_all = np.concatenate(dstl_parts)
        TOT_G = len(idx_all) // 128

        # device layouts
        idx_tiled = np.tile(idx_all.reshape(-1, 16).T, (8, 1)).copy()
        # dstl: [128 edge-slot, TOT_G] bf16
        dstl_tiled = np.ascontiguousarray(
            dstl_all.reshape(TOT_G, 128).T).astype(ml_dtypes.bfloat16)
        d = np.ones(CHP, np.float32)
        d[:CH] = dis[c * CH:(c + 1) * CH]
        dis_win = np.ascontiguousarray(d.reshape(NWIN, 128).T)
        cores.append(dict(idx=idx_tiled, dstl=dstl_tiled, dis_win=dis_win))
    return G, cores, CH, NWIN, CHP, NWA


# ---------------------------------------------------------------------------
# Bass program
# ---------------------------------------------------------------------------
def build_program(DIN, DRS, DTS, G, NWIN, CHP, NWA, TOT_IDX, TOT_G,
                  G_CAP, BLK_CAP, biases_nonzero):
    """DRS: real per-layer output dims [256,256,64];
    DTS: padded table dims [256,256,128]."""
    from concourse import bacc, bass, tile, mybir

    f32 = mybir.dt.float32
    bf16 = mybir.dt.bfloat16
    i16 = mybir.dt.int16
    ADD = mybir.AluOpType.add
    EQ = mybir.AluOpType.is_equal
    CPY = mybir.ActivationFunctionType.Copy
    NL = len(DRS)
    NBLK = _cdiv(NWIN, W_BLK)

    nc = bacc.Bacc("TRN2", target_bir_lowering=False, debug=False,
                   enable_asserts=False, num_devices=NC_CORES,
                   num_swdge_queues=4, dynamic_dma_scratch_size=32768)

    # --- I/O tensors ---
    xT_d = nc.dram_tensor("xT", [DIN, CHP], bf16, kind="ExternalInput")
    W_d = [nc.dram_tensor(f"W{i}", [DRS[i - 1] if i else DIN, DRS[i]], bf16,
                          kind="ExternalInput") for i in range(NL)]
    bias_d = [nc.dram_tensor(f"bias{i}", [128, DRS[i]], f32,
                             kind="ExternalInput") for i in range(NL)]
    idx_d = nc.dram_tensor("idx", [128, TOT_IDX // 16], i16, kind="ExternalInput")
    dstl_d = nc.dram_tensor("dstl", [128, TOT_G], bf16, kind="ExternalInput")
    iotag_d = nc.dram_tensor("iotag", [128, 128 * G_CAP], bf16,
                             kind="ExternalInput")
    dis_d = nc.dram_tensor("dis_win", [128, NWIN], f32, kind="ExternalInput")
    ident_d = nc.dram_tensor("ident", [128, 128], bf16, kind="ExternalInput")
    out_d = nc.dram_tensor("out", [CHP, DRS[-1]], f32, kind="ExternalOutput")

    with tile.TileContext(nc) as tc:
        with (
            tc.tile_pool(name="const", bufs=1) as constp,
            tc.tile_pool(name="ht", bufs=1) as htp,
            tc.tile_pool(name="wts", bufs=2) as wtsp,
            tc.tile_pool(name="zs", bufs=3) as zsp,
            tc.tile_pool(name="zw", bufs=3) as zwp,
            tc.tile_pool(name="gath", bufs=4) as gathp,
            tc.tile_pool(name="ohA", bufs=3) as ohpA,
            tc.tile_pool(name="ohB", bufs=3) as ohpB,
            tc.tile_pool(name="epi", bufs=3) as epip,
            tc.tile_pool(name="psz", bufs=2, space="PSUM") as pszp,
            tc.tile_pool(name="psw", bufs=3, space="PSUM") as pswp,
            tc.tile_pool(name="pst", bufs=2, space="PSUM") as pstp,
            tc.tile_pool(name="dram", bufs=1, space="DRAM") as dramp,
        ):
            # --- persistent SBUF constants ---
            idx_t = constp.tile([128, TOT_IDX // 16], i16, tag="idx")
            nc.sync.dma_start(idx_t[:], idx_d[:])
            dstl_t = constp.tile([128, TOT_G], bf16, tag="dstl")
            nc.sync.dma_start(dstl_t[:], dstl_d[:])
            iotag_t = constp.tile([128, 128 * G_CAP], bf16, tag="iotag")
            nc.sync.dma_start(iotag_t[:], iotag_d[:])
            dis_t = constp.tile([128, NWIN], f32, tag="dis")
            nc.sync.dma_start(dis_t[:], dis_d[:])
            ident_t = constp.tile([128, 128], bf16, tag="ident")
            nc.sync.dma_start(ident_t[:], ident_d[:])
            # one gpsimd register per distinct gather size, set once
            sizes = set()
            for b in range(NBLK):
                for hh in range(2):
                    gsec = int(G[b * W_BLK:(b + 1) * W_BLK, hh].sum())
                    while gsec > 0:
                        gc = min(GMAX, gsec)
                        sizes.add(gc * 128)
                        gsec -= gc
            nreg = {}
            for v in sorted(sizes):
                r = nc.gpsimd.alloc_register(f"nidx{v}")
                nc.gpsimd.reg_mov(r, v)
                nreg[v] = r
            bias_t = []
            for i in range(NL):
                if biases_nonzero[i]:
                    bt = constp.tile([128, DRS[i]], f32, tag=f"bias{i}")
                    nc.sync.dma_start(bt[:], bias_d[i][:])
                    bias_t.append(bt)
                else:
                    bias_t.append(None)

            # --- H^T SBUF-resident double buffer: [k][128, CHP] bf16 ---
            KT0 = DIN // 128
            ht_cur = [htp.tile([128, CHP], bf16, tag=f"htA{k}",
                               name=f"htA{k}") for k in range(KT0)]
            for k in range(KT0):
                nc.sync.dma_start(ht_cur[k][:], xT_d[k * 128:(k + 1) * 128, :])
            ht_nxt = [htp.tile([128, CHP], bf16, tag=f"htB{k}",
                               name=f"htB{k}") for k in range(KT0)]

            zs_own = [dramp.tile([CHP, DTS[i]], bf16, tag=f"zso{i}",
                                 name=f"zs_own{i}") for i in range(NL)]
            HA = NWA * 128
            HB = CHP - HA
            zs_fullA = [dramp.tile([NC_CORES * HA, DTS[i]], bf16,
                                   tag=f"zsfA{i}", addr_space="Shared",
                                   name=f"zs_fullA{i}") for i in range(NL)]
            zs_fullB = [dramp.tile([NC_CORES * HB, DTS[i]], bf16,
                                   tag=f"zsfB{i}", addr_space="Shared",
                                   name=f"zs_fullB{i}") for i in range(NL)]

            RG = [list(range(NC_CORES))]

            def emit_z_tile(li, t, lhs_tiles):
                """Z-matmul + scale for node-tile t of layer li."""
                Dr = DRS[li]
                Dt = DTS[li]
                KT = DIN // 128 if li == 0 else DRS[li - 1] // 128
                psz = pszp.tile([128, Dr], f32, tag="psz", name="psz")
                for k in range(KT):
                    nc.tensor.matmul(psz[:],
                                     lhs_tiles[k][:, t * 128:(t + 1) * 128],
                                     wk[li][k][:],
                                     start=(k == 0), stop=(k == KT - 1))
                zst = zsp.tile([128, Dt], bf16, tag="zst", name="zst")
                nc.scalar.activation(zst[:, :Dr], psz[:], CPY,
                                     scale=dis_t[:, t:t + 1])
                if Dr < Dt:
                    nc.vector.memset(zst[:, Dr:], 0.0)
                nc.sync.dma_start(zs_own[li][t * 128:(t + 1) * 128, :],
                                  zst[:])

            def emit_ag(li, half):
                if half == 0:
                    nc.gpsimd.collective_compute(
                        "AllGather", bass.mybir.AluOpType.bypass,
                        replica_groups=RG,
                        ins=[zs_own[li][:HA, :]],
                        outs=[zs_fullA[li].opt()])
                else:
                    nc.gpsimd.collective_compute(
                        "AllGather", bass.mybir.AluOpType.bypass,
                        replica_groups=RG,
                        ins=[zs_own[li][HA:, :]],
                        outs=[zs_fullB[li].opt()])

            # weight tiles for every layer, loaded up front (small)
            wk = []
            for li in range(NL):
                KT = DIN // 128 if li == 0 else DRS[li - 1] // 128
                wkl = []
                for k in range(KT):
                    wt_ = wtsp.tile([128, DRS[li]], bf16, tag=f"wk{li}_{k}",
                                    name=f"wk{li}_{k}")
                    nc.sync.dma_start(wt_[:], W_d[li][k * 128:(k + 1) * 128, :])
                    wkl.append(wt_)
                wk.append(wkl)

            # ---- layer-0 z-phase + split AllGather ----
            for t in range(NWIN):
                emit_z_tile(0, t, ht_cur)
                if t == NWA - 1:
                    emit_ag(0, 0)
            emit_ag(0, 1)

            # ---- main loop: gather/aggregate layer li; z + AG of li+1
            #      interleaved so the collectives hide under the gathers ----
            for li in range(NL):
                Dr = DRS[li]
                Dt = DTS[li]
                ioff16 = 0   # running idx offset (int16 cols)
                gbase = 0    # running group (dstl column) offset
                qrr = 0
                for b in range(NBLK):
                    ws = list(range(b * W_BLK, min((b + 1) * W_BLK, NWIN)))
                    Gls = [int(G[w, 0]) for w in ws]
                    Ghs = [int(G[w, 1]) for w in ws]
                    AGb, BGb = sum(Gls), sum(Ghs)
                    TG = AGb + BGb
                    wt = gathp.tile([128, BLK_CAP, Dt], bf16, tag="gather",
                                    name="wt")
                    for half, gsec, goff in ((0, AGb, 0), (1, BGb, AGb)):
                        tbl = zs_fullA[li] if half == 0 else zs_fullB[li]
                        g0 = 0
                        while g0 < gsec:
                            gc = min(GMAX, gsec - g0)
                            nc.gpsimd.dma_gather(
                                wt[:, goff + g0:goff + g0 + gc, :],
                                tbl[:],
                                idx_t[:, ioff16:ioff16 + gc * 8],
                                num_idxs=gc * 128,
                                num_idxs_reg=nreg[gc * 128],
                                elem_size=Dt,
                                queue_num=qrr % 4)
                            qrr += 1
                            ioff16 += gc * 8
                            g0 += gc
                    aoff, boff = 0, 0
                    for wi_i, w in enumerate(ws):
                        Gl, Gh = Gls[wi_i], Ghs[wi_i]
                        # one-hot per half: oh[p, j, g] = (dstl[p, g] == j)
                        ohA = None
                        if Gl:
                            ohA = ohpA.tile([128, 128, Gl], bf16, tag="ohA",
                                            name="ohA")
                            nc.vector.tensor_tensor(
                                ohA[:],
                                dstl_t[:, gbase + aoff:gbase + aoff + Gl]
                                    .unsqueeze(1).broadcast_to((128, 128, Gl)),
                                iotag_t[:].rearrange("p (j g) -> p j g",
                                                     g=G_CAP)[:, :, :Gl],
                                op=EQ,
                            )
                        ohB = None
                        if Gh:
                            ohB = ohpB.tile([128, 128, Gh], bf16, tag="ohB",
                                            name="ohB")
                            nc.vector.tensor_tensor(
                                ohB[:],
                                dstl_t[:, gbase + AGb + boff:
                                       gbase + AGb + boff + Gh]
                                    .unsqueeze(1).broadcast_to((128, 128, Gh)),
                                iotag_t[:].rearrange("p (j g) -> p j g",
                                                     g=G_CAP)[:, :, :Gh],
                                op=EQ,
                            )
                        # z rows for the self term (bf16, local DRAM)
                        zw = zwp.tile([128, Dr], bf16, tag="zw", name="zw")
                        nc.sync.dma_start(
                            zw[:], zs_own[li][w * 128:(w + 1) * 128, :Dr])
                        psw = pswp.tile([128, Dt], f32, tag="psw", name="psw")
                        nmm = Gl + Gh + 1
                        k = 0
                        for g in range(Gl):
                            nc.tensor.matmul(psw[:], ohA[:, :, g],
                                             wt[:, aoff + g, :],
                                             start=(k == 0), stop=False)
                            k += 1
                        for g in range(Gh):
                            nc.tensor.matmul(psw[:], ohB[:, :, g],
                                             wt[:, AGb + boff + g, :],
                                             start=(k == 0), stop=False)
                            k += 1
                        # self term: psw[j] += zs_own[w*128+j]  (the
                        # epilogue's dis[dst] scale turns this into the
                        # dis^2 z self contribution)
                        nc.tensor.matmul(psw[:, :Dr], ident_t[:], zw[:],
                                         start=(k == 0), stop=True)
                        aoff += Gl
                        boff += Gh
                        # epilogue: relu(dis * acc) (+bias if nonzero)
                        if bias_t[li] is not None:
                            t1 = epip.tile([128, Dr], f32, tag="t1",
                                           name="t1")
                            nc.vector.tensor_tensor(t1[:], psw[:, :Dr],
                                                    bias_t[li][:], op=ADD)
                            src_ap = t1[:]
                        else:
                            src_ap = psw[:, :Dr]
                        if li < NL - 1:
                            h2 = epip.tile([128, Dr], bf16, tag="h2",
                                           name="h2")
                            nc.scalar.activation(
                                h2[:], src_ap,
                                bass.mybir.ActivationFunctionType.Relu,
                                scale=dis_t[:, w:w + 1])
                            for k2 in range(Dr // 128):
                                pst = pstp.tile([128, 128], bf16, tag="pst",
                                                name="pst")
                                nc.tensor.transpose(
                                    pst[:], h2[:, k2 * 128:(k2 + 1) * 128],
                                    ident_t[:])
                                nc.vector.tensor_copy(
                                    ht_nxt[k2][:, w * 128:(w + 1) * 128],
                                    pst[:])
                            # interleaved z for layer li+1 (its H^T tile-w is
                            # ready now); AG halves fire mid-phase
                            emit_z_tile(li + 1, w, ht_nxt)
                            if w == NWA - 1:
                                emit_ag(li + 1, 0)
                        else:
                            h2o = epip.tile([128, Dr], f32, tag="h2o",
                                            name="h2o")
                            nc.scalar.activation(h2o[:], src_ap, CPY,
                                                 scale=dis_t[:, w:w + 1])
                            nc.sync.dma_start(out_d[w * 128:(w + 1) * 128, :],
                                              h2o[:])
                    gbase += TG
                if li < NL - 1:
                    emit_ag(li + 1, 1)
                    ht_cur, ht_nxt = ht_nxt, ht_cur
    nc.compile()
    return nc


# ---------------------------------------------------------------------------
# Entry point
# ---------------------------------------------------------------------------
def kernel(x, edge_index, W1, b1, W2, b2, W3, b3):
    from concourse.bass_utils import run_bass_kernel_spmd
    import ml_dtypes

    bfnp = ml_dtypes.bfloat16
    x = np.asarray(x, dtype=np.float32)
    Ws = [np.asarray(w, dtype=np.float32) for w in (W1, W2, W3)]
    bs = [np.asarray(b, dtype=np.float32) for b in (b1, b2, b3)]

    N, DIN = x.shape
    DRS = [w.shape[1] for w in Ws]
    DTS = [max(d, 128) for d in DRS]
    NL = 3

    G, cores, CH, NWIN, CHP, NWA = preprocess(edge_index, N)
    TOT_IDX = cores[0]["idx"].shape[1] * 16
    TOT_G = cores[0]["dstl"].shape[1]
    G_CAP = int(G.max())
    NBLK = _cdiv(NWIN, W_BLK)
    BLK_CAP = max(
        int(G[b * W_BLK:(b + 1) * W_BLK].sum()) for b in range(NBLK))
    biases_nonzero = [bool(np.any(b != 0)) for b in bs]

    nc = build_program(DIN, DRS, DTS, G, NWIN, CHP, NWA, TOT_IDX, TOT_G,
                       G_CAP, BLK_CAP, biases_nonzero)

    ident = np.eye(128, dtype=bfnp)
    # iotag[p, j*G_CAP + g] = j
    iotag = np.tile(np.repeat(np.arange(128), G_CAP).astype(bfnp), (128, 1))
    in_maps = []
    for c in range(NC_CORES):
        xT = np.zeros((DIN, CHP), bfnp)
        xT[:, :CH] = x[c * CH:(c + 1) * CH].T.astype(bfnp)
        m = {
            "xT": xT,
            "idx": cores[c]["idx"],
            "dstl": cores[c]["dstl"],
            "iotag": iotag,
            "dis_win": cores[c]["dis_win"],
            "ident": ident,
        }
        for i in range(NL):
            m[f"W{i}"] = Ws[i].astype(bfnp)
            m[f"bias{i}"] = np.tile(bs[i][None, :], (128, 1))
        in_maps.append(m)

    trace = bool(int(os.environ.get("GCN_TRACE", "0")))
    res = run_bass_kernel_spmd(nc, in_maps, core_ids=list(range(NC_CORES)),
                               trace=trace)
    kernel.last_results = res
    out = np.concatenate([res.results[c]["out"][:CH] for c in range(NC_CORES)],
                         axis=0)
    return out.astype(np.float32)
